# revision 12
# baseline (speedup 1.0000x reference)
"""Trainium2 Bass kernel for nn_DialogueGCNModel (DialogueGCN forward).

Strategy (data-parallel over dialogues, 4 dialogues per core, fp8 DoubleRow):
  - All large matmuls run in fp8e4 with MatmulPerfMode.DoubleRow (2x128
    contraction per instruction at 0.5 cycles/output-column).
  - The RGCN relational gather/scatter is factorized: etype = 4*spk_src +
    2*spk_dst + dir, so  agg = sum_{b,d} W'_d^T Y_{b,d}  with two SHARED
    banded direction masks W_d (invd and the out1 copy scale folded in).
    Y (the src-speaker select of xr) is built inside one psum group by
    matmul'ing a host-premasked copy xu = u0*x against (Wlo-Whi) and x
    against Whi; the dst-speaker select is one copy_predicated between two
    psum halves, and the root term accumulates into the same psum.
  - Every fp8 tensor is scaled on host via rigorous norm bounds so no cast
    can overflow; all unscale factors ride as [128,1] scalar-AP data so the
    compiled program is input-independent.
  - Host pre-packs every operand in its exact SBUF layout -> every DMA is a
    flat full-bandwidth copy, ordered by first use. Dialogues are paired in
    the psum free dim so psum->SBUF copies run at [128,512] granularity.

kernel(**inputs) takes FULL inputs, runs 8-core SPMD via
bass_utils.run_bass_kernel_spmd, returns the FULL (8192, 7) f32 output.
"""

import numpy as np
import ml_dtypes

BF16 = ml_dtypes.bfloat16
FP8 = ml_dtypes.float8_e4m3    # IEEE e4m3: max finite 240

B, L, D, H, R, NB, C = 32, 256, 1024, 128, 8, 30, 7
WP, WF = 10, 10
MEM = D + H            # 1152
N = B * L              # 8192
NCORES = 8
DPC = B // NCORES      # dialogues per core = 4
NLOC = DPC * L         # nodes per core = 1024
NT = NLOC // 128       # node tiles per core = 8
KT = D // 128          # contraction tiles over D = 8
MT = MEM // 128        # tiles over MEM = 9
MP = MT + 1            # padded to 1280 for DoubleRow pairing

FP8MAX = 112.0

_cache = {}


def _build_program(use_mask):
    import concourse.bacc as bacc
    import concourse.tile as tile
    import concourse.mybir as mybir
    import concourse.bass as bass
    from concourse.masks import make_identity

    dt = mybir.dt
    f32, bf16, fp8 = dt.float32, dt.bfloat16, dt.float8e4
    AX = mybir.AxisListType.X
    AF = mybir.ActivationFunctionType
    OP = mybir.AluOpType
    DR = mybir.MatmulPerfMode.DoubleRow

    nc = bacc.Bacc("TRN2", target_bir_lowering=False, debug=False,
                   num_devices=NCORES)

    dram = nc.dram_tensor
    xt_d = dram("xt", [128, KT, NLOC], fp8, kind="ExternalInput")
    xu_d = dram("xu", [128, KT, NLOC], fp8, kind="ExternalInput")
    wrd_d = dram("wrd", [128, KT, 512], fp8, kind="ExternalInput")
    wrh_d = dram("wrh", [128, KT, 512], fp8, kind="ExternalInput")
    wr1_d = dram("wr1", [128, KT, H], fp8, kind="ExternalInput")
    aggm_d = dram("aggm", [128, 2, 2, L], fp8, kind="ExternalInput")
    b01_d = dram("b01", [128, 2, L], fp8, kind="ExternalInput")
    w2_d = dram("w2", [128, 2, H], bf16, kind="ExternalInput")
    wt_d = dram("wt", [128, MT, MP, 128], fp8, kind="ExternalInput")
    wlin_d = dram("wlin", [128, MP, H], fp8, kind="ExternalInput")
    wfc_d = dram("wfc", [128, C], bf16, kind="ExternalInput")
    bfc_d = dram("bfc", [1, C], bf16, kind="ExternalInput")
    vb_d = dram("vb", [DPC, L], dt.uint8, kind="ExternalInput")
    sc_d = dram("sc", [128, 16], f32, kind="ExternalInput")
    bias_d = dram("bias", [128, 12], f32, kind="ExternalInput")
    if use_mask:
        um_d = dram("um", [128, DPC, 2, L], f32, kind="ExternalInput")
    out_d = dram("out", [NLOC, C], f32, kind="ExternalOutput")

    with tile.TileContext(nc) as tc:
        from contextlib import ExitStack
        with ExitStack() as ctx:
            consts = ctx.enter_context(tc.tile_pool(name="consts", bufs=1))
            work = ctx.enter_context(tc.tile_pool(name="work", bufs=6))
            ps = ctx.enter_context(tc.tile_pool(name="ps", bufs=6, space="PSUM"))
            pst = ctx.enter_context(tc.tile_pool(name="pst", bufs=1, space="PSUM"))

            dma = nc.sync.dma_start
            mm = nc.tensor.matmul

            xt = consts.tile([128, KT, NLOC], fp8)
            xu = consts.tile([128, KT, NLOC], fp8)
            wrd = consts.tile([128, KT, 512], fp8)
            wrh = consts.tile([128, KT, 512], fp8)
            wr1 = consts.tile([128, KT, H], fp8)
            aggm = consts.tile([128, 2, 2, L], fp8)
            b01 = consts.tile([128, 2, L], fp8)
            w2 = consts.tile([128, 2, H], bf16)
            wt = consts.tile([128, MT, MP, 128], fp8)
            wlin = consts.tile([128, MP, H], fp8)
            wfc = consts.tile([128, C], bf16)
            bfc = consts.tile([1, C], bf16)
            vb = consts.tile([128, DPC, L], dt.uint8)
            sc = consts.tile([128, 16], f32)
            bias = consts.tile([128, 12], f32)
            if use_mask:
                um = consts.tile([128, DPC, 2, L], f32)

            Y = consts.tile([128, NT, 512], fp8)
            rhs23 = consts.tile([128, 2, DPC, L], bf16)  # slot0 nbT, slot1 out1T
            o1n = consts.tile([128, DPC, 2, H], fp8)
            mtail = consts.tile([128, 2, DPC, L], fp8)   # slot0 out2T, slot1 zero
            XcT = consts.tile([128, MP, DPC, L], fp8)    # slot MT zero
            G8 = consts.tile([128, DPC, 2, H], fp8)
            alphaT = consts.tile([128, DPC, 2, L], fp8)
            z = consts.tile([128, DPC, 2, L], f32)
            hidT = consts.tile([128, DPC, L], bf16)
            o_all = consts.tile([128, DPC, 2, 8], f32)

            # DMAs in first-use order (SP engine queue)
            dma(out=wrh, in_=wrh_d[:])
            dma(out=xt[:, :, 0:128], in_=xt_d[:, :, 0:128])
            dma(out=wrd, in_=wrd_d[:])
            dma(out=xu[:, :, 0:128], in_=xu_d[:, :, 0:128])
            dma(out=xt[:, :, 128:512], in_=xt_d[:, :, 128:512])
            dma(out=xu[:, :, 128:512], in_=xu_d[:, :, 128:512])
            dma(out=xt[:, :, 512:NLOC], in_=xt_d[:, :, 512:NLOC])
            dma(out=xu[:, :, 512:NLOC], in_=xu_d[:, :, 512:NLOC])
            dma(out=sc, in_=sc_d[:])
            dma(out=bias, in_=bias_d[:])
            dma(out=aggm, in_=aggm_d[:])
            dma(out=wr1, in_=wr1_d[:])
            dma(out=b01, in_=b01_d[:])
            dma(out=w2, in_=w2_d[:])
            dma(out=wt, in_=wt_d[:])
            dma(out=wlin, in_=wlin_d[:])
            dma(out=wfc, in_=wfc_d[:])
            dma(out=bfc, in_=bfc_d[:])
            if use_mask:
                dma(out=um, in_=um_d[:])

            def bcast(dst, src_ap):
                bc = bass.AP(tensor=src_ap.tensor, offset=src_ap.offset,
                             ap=[[0, 128]] + list(src_ap.ap))
                nc.gpsimd.dma_start(out=dst, in_=bc)

            bcast(vb, vb_d[:])

            ones_row = consts.tile([1, 128], bf16)
            nc.vector.memset(ones_row, 1.0)
            ident = consts.tile([128, 128], bf16)
            make_identity(nc, ident)
            nc.gpsimd.memset(mtail[:, 1, :, :], 0.0)
            nc.gpsimd.memset(XcT[:, MT, :, :], 0.0)

            # PE clock warm-up during the DMA lead-in
            warm_in = consts.tile([128, 512], bf16)
            nc.vector.memset(warm_in[:, 0:128], 0.0)
            warm = ps.tile([128, 2, 256], f32, tag="mm")
            for _ in range(16):
                mm(warm, lhsT=warm_in[:, 0:128], rhs=warm_in, start=True,
                   stop=True, skip_group_check=True)

            # ---- stage 1: Y psum = x @ Whi (+ u0*x @ (Wlo-Whi)) ----
            # sliding window: hi-group of tile i+1 is emitted before the
            # diff-group of tile i, giving the xu DMA extra slack
            p1s = {}

            def emit_hi(i):
                nsl = slice(i * 128, (i + 1) * 128)
                p1 = ps.tile([128, 2, 256], f32, tag="mm")
                p1s[i] = p1
                for k in range(KT // 2):
                    mm(p1[:, :, :], lhsT=xt[:, 2 * k:2 * k + 2, nsl],
                       rhs=wrh[:, 2 * k:2 * k + 2, :], start=(k == 0),
                       stop=False, perf_mode=DR, skip_group_check=True)

            def emit_diff(i):
                nsl = slice(i * 128, (i + 1) * 128)
                p1 = p1s.pop(i)
                for k in range(KT // 2):
                    mm(p1[:, :, :], lhsT=xu[:, 2 * k:2 * k + 2, nsl],
                       rhs=wrd[:, 2 * k:2 * k + 2, :], start=False,
                       stop=(k == KT // 2 - 1), perf_mode=DR,
                       skip_group_check=True)
                if i % 2 == 0:
                    nc.scalar.activation(Y[:, i, :], p1[:, :, :], AF.Copy,
                                         scale=sc[:, 0:1])
                else:
                    nc.vector.tensor_scalar(out=Y[:, i, :], in0=p1[:, :, :],
                                            scalar1=sc[:, 0:1], scalar2=None,
                                            op0=OP.mult)

            emit_hi(0)
            for i in range(1, NT):
                emit_hi(i)
                emit_diff(i - 1)
            emit_diff(NT - 1)

            # ---- stage 2: agg (pred-merge) + root, dialogue-paired ----
            pas, pbs = [], []
            for dp in range(DPC // 2):
                pa = ps.tile([128, 2, 256], f32, tag="mm")
                pb = ps.tile([128, 2, 256], f32, tag="mm")
                pas.append(pa)
                pbs.append(pb)
                for h in range(2):
                    d = 2 * dp + h
                    for dd in range(2):
                        mm(pa[:, h, :],
                           lhsT=Y[:, 2 * d:2 * d + 2, dd * H:(dd + 1) * H],
                           rhs=aggm[:, dd, :, :], start=(dd == 0), stop=False,
                           perf_mode=DR, skip_group_check=True)
                    for dd in range(2):
                        mm(pb[:, h, :],
                           lhsT=Y[:, 2 * d:2 * d + 2,
                                  (2 + dd) * H:(3 + dd) * H],
                           rhs=aggm[:, dd, :, :], start=(dd == 0),
                           stop=(dd == 1), perf_mode=DR, skip_group_check=True)
            for dp in range(DPC // 2):
                nc.vector.copy_predicated(pas[dp][:, :, :],
                                          vb[:, 2 * dp:2 * dp + 2, :],
                                          pbs[dp][:, :, :])
            for dp in range(DPC // 2):
                for h in range(2):
                    d = 2 * dp + h
                    for k in range(KT // 2):
                        mm(pas[dp][:, h, :], lhsT=wr1[:, 2 * k:2 * k + 2, :],
                           rhs=xt[:, 2 * k:2 * k + 2, d * L:(d + 1) * L],
                           start=False, stop=(k == KT // 2 - 1),
                           perf_mode=DR, skip_group_check=True)
            for dp in range(DPC // 2):
                nc.vector.tensor_scalar(
                    out=rhs23[:, 1, 2 * dp:2 * dp + 2, :], in0=pas[dp][:, :, :],
                    scalar1=bias[:, 0:1], scalar2=None, op0=OP.add)

            # out1 transposes: [h, s] -> [s, h], 4 per d-pair in one psum
            for dp in range(DPC // 2):
                tp = pst.tile([128, 2, 256], bf16, tag="tr")
                for h in range(2):
                    d = 2 * dp + h
                    for st in range(2):
                        nc.tensor.transpose(
                            tp[:, h, st * 128:(st + 1) * 128],
                            rhs23[:, 1, d, st * 128:(st + 1) * 128], ident)
                nc.vector.tensor_copy(o1n[:, 2 * dp:2 * dp + 2, :, :], tp)

            # ---- stage 3: GraphConv, dialogue-paired psums ----
            for dp in range(DPC // 2):
                p2 = ps.tile([128, 2, 256], f32, tag="mm")
                for h in range(2):
                    mm(p2[:, h, :], lhsT=o1n[:, 2 * dp + h, :, :], rhs=b01,
                       start=True, stop=True, perf_mode=DR,
                       skip_group_check=True)
                nc.vector.tensor_scalar(
                    out=rhs23[:, 0, 2 * dp:2 * dp + 2, :], in0=p2[:, :, :],
                    scalar1=0.03125, scalar2=None, op0=OP.mult)
            for dp in range(DPC // 2):
                p3 = ps.tile([128, 2, 256], f32, tag="mm")
                for h in range(2):
                    mm(p3[:, h, :], lhsT=w2[:, 0, :],
                       rhs=rhs23[:, 0, 2 * dp + h, :],
                       start=True, stop=False, skip_group_check=True)
                    mm(p3[:, h, :], lhsT=w2[:, 1, :],
                       rhs=rhs23[:, 1, 2 * dp + h, :],
                       start=False, stop=True, skip_group_check=True)
                nc.scalar.activation(mtail[:, 0, 2 * dp:2 * dp + 2, :],
                                     p3[:, :, :], AF.Identity,
                                     bias=bias[:, 1:2], scale=sc[:, 1:2])

            # M^T pair accessor for DoubleRow pair j of dialogue d
            def mpair(j, d, c0=0, c1=L):
                if j < 4:
                    return xt[:, 2 * j:2 * j + 2, d * L + c0:d * L + c1]
                return mtail[:, :, d, c0:c1]

            # ---- stages 5-6 pipelined per dialogue-pair: Xc -> scores
            # -> tanh/exp overlap the next pair's Xc psums on PE ----
            alfs = {}
            ln_dummy_src = consts.tile([1, 1], f32)
            ln_dummy = consts.tile([1, 1], f32)
            nc.vector.memset(ln_dummy_src, 1.0)
            for dp in range(DPC // 2):
                for n2 in range(MT):
                    p4 = ps.tile([128, 2, 256], f32, tag="mm")
                    for h in range(2):
                        d = 2 * dp + h
                        for j in range(5):
                            mm(p4[:, h, :], lhsT=wt[:, n2, 2 * j:2 * j + 2, :],
                               rhs=mpair(j, d), start=(j == 0), stop=(j == 4),
                               perf_mode=DR, skip_group_check=True)
                    if n2 % 2 == 0:
                        nc.scalar.activation(
                            XcT[:, n2, 2 * dp:2 * dp + 2, :], p4[:, :, :],
                            AF.Identity, bias=bias[:, 2 + n2:3 + n2],
                            scale=sc[:, 2 + n2:3 + n2])
                    else:
                        nc.vector.tensor_scalar(
                            out=XcT[:, n2, 2 * dp:2 * dp + 2, :],
                            in0=p4[:, :, :], scalar1=sc[:, 2 + n2:3 + n2],
                            scalar2=bias[:, 2 + n2:3 + n2],
                            op0=OP.mult, op1=OP.add)
                # scores + tanh for this pair
                for h in range(2):
                    d = 2 * dp + h
                    p5 = ps.tile([128, 2, 256], f32, tag="mm")
                    for tt in range(2):
                        for j in range(5):
                            mm(p5[:, tt, :],
                               lhsT=XcT[:, 2 * j:2 * j + 2, d,
                                        tt * 128:(tt + 1) * 128],
                               rhs=mpair(j, d), start=(j == 0), stop=(j == 4),
                               perf_mode=DR, skip_group_check=True)
                    if use_mask:
                        nc.vector.scalar_tensor_tensor(
                            out=z[:, d, :, :], in0=p5[:, :, :],
                            scalar=sc[:, 11:12], in1=um[:, d, :, :],
                            op0=OP.mult, op1=OP.mult)
                        nc.scalar.activation(z[:, d, :, :], z[:, d, :, :],
                                             AF.Tanh)
                    else:
                        nc.scalar.activation(z[:, d, :, :], p5[:, :, :],
                                             AF.Tanh, scale=sc[:, 11:12])
                # G psums for this pair (keeps PE fed during tanh/exp)
                for h in range(2):
                    d = 2 * dp + h
                    pg = ps.tile([128, 2, 256], f32, tag="mm")
                    for st in range(2):
                        for j in range(5):
                            mm(pg[:, st, 0:128],
                               lhsT=mpair(j, d, st * 128, (st + 1) * 128),
                               rhs=wlin[:, 2 * j:2 * j + 2, :], start=(j == 0),
                               stop=(j == 4), perf_mode=DR,
                               skip_group_check=True)
                    nc.vector.tensor_scalar(out=G8[:, d, :, :],
                                            in0=pg[:, :, 0:128],
                                            scalar1=sc[:, 13:14], scalar2=None,
                                            op0=OP.mult)
                if dp == DPC // 2 - 1:
                    # hoist the exp+ln table switch out of the tail: after the
                    # last Tanh, a dummy Ln forces the natural_log_exp set
                    nc.scalar.activation(ln_dummy, ln_dummy_src, AF.Ln)
                # softmax exp/recip/alpha for this pair
                for h in range(2):
                    d = 2 * dp + h
                    for tt in range(2):
                        zz = z[:, d, tt, :]
                        ssum = work.tile([128, 1], f32, tag=f"ssum{d}{tt}")
                        nc.scalar.activation(zz, zz, AF.Exp, accum_out=ssum)
                        if use_mask:
                            nc.vector.tensor_mul(zz, zz, um[:, d, 1, :])
                            nc.vector.reduce_sum(out=ssum, in_=zz, axis=AX)
                        rinv = work.tile([128, 1], f32, tag=f"rinv{d}{tt}")
                        nc.vector.reciprocal(rinv, ssum)
                        alf = work.tile([128, L], bf16, tag=f"alf{d}{tt}")
                        nc.gpsimd.tensor_scalar(out=alf, in0=zz, scalar1=rinv,
                                                scalar2=64.0, op0=OP.mult,
                                                op1=OP.mult)
                        alfs[(d, tt)] = alf

            # alpha transposes (PE) + copies (DVE)
            for d in range(DPC):
                tpa = pst.tile([128, 2, 256], bf16, tag="tr")
                for st in range(2):
                    for tt in range(2):
                        nc.tensor.transpose(
                            tpa[:, st, tt * 128:(tt + 1) * 128],
                            alfs[(d, tt)][:, st * 128:(st + 1) * 128], ident)
                if d % 2 == 0:
                    nc.vector.tensor_copy(alphaT[:, d, :, :], tpa)
                else:
                    nc.scalar.copy(alphaT[:, d, :, :], tpa)
            # hidden psums (PE) + Relu (ACT)
            for d in range(DPC):
                p7 = ps.tile([128, 2, 256], f32, tag="mm")
                mm(p7[:, 0, :], lhsT=G8[:, d, :, :], rhs=alphaT[:, d, :, :],
                   start=True, stop=True, perf_mode=DR, skip_group_check=True)
                nc.scalar.activation(hidT[:, d, :], p7[:, 0, :], AF.Relu,
                                     bias=bias[:, 11:12], scale=sc[:, 12:13])
            # logits psums (PE) + Exp (ACT)
            s7s = {}
            for d in range(DPC):
                for tt in range(2):
                    p8 = ps.tile([128, 2, 256], f32, tag="mm")
                    mm(p8[:, 0, :C], lhsT=hidT[:, d, tt * 128:(tt + 1) * 128],
                       rhs=wfc, start=True, stop=False, skip_group_check=True)
                    mm(p8[:, 0, :C], lhsT=ones_row, rhs=bfc, start=False,
                       stop=True, skip_group_check=True)
                    e7 = work.tile([128, 8], f32, tag="e7")
                    s7 = work.tile([128, 1], f32, tag=f"s7_{d}{tt}")
                    nc.scalar.activation(e7[:, :C], p8[:, 0, :C], AF.Exp,
                                         accum_out=s7)
                    nc.vector.tensor_scalar(
                        out=o_all[:, d, tt, :C], in0=p8[:, 0, :C],
                        scalar1=0.0, scalar2=None, op0=OP.add)
                    s7s[(d, tt)] = s7
            # Ln (one table switch) + subtract (DVE)
            for d in range(DPC):
                for tt in range(2):
                    ls7 = work.tile([128, 1], f32, tag=f"ls7_{d}{tt}")
                    nc.scalar.activation(ls7, s7s[(d, tt)], AF.Ln)
                    nc.vector.tensor_scalar(
                        out=o_all[:, d, tt, :C], in0=o_all[:, d, tt, :C],
                        scalar1=ls7, scalar2=None, op0=OP.subtract)

            dma(out=out_d[:].rearrange("(d tt p) c -> p d tt c", d=DPC, tt=2),
                in_=o_all[:, :, :, 0:C])

    nc.compile()
    return nc


def _colnorm_max(a):
    return float(np.sqrt((np.asarray(a, np.float64) ** 2).sum(0)).max())


def _rownorm_max(a):
    return float(np.sqrt((np.asarray(a, np.float64) ** 2).sum(-1)).max())


def _q8(a, name=""):
    a = np.asarray(a, np.float32)
    mx = float(np.abs(a).max()) if a.size else 0.0
    assert mx <= 239.0, f"fp8 overflow in {name}: {mx}"
    return a.astype(FP8)


def prep_inputs(x, edge_src, edge_dst, edge_type, umask, basis, comp,
                w_root1, b1, w_rel2, b_rel2, w_root2, w_t, b_t,
                w_lin, b_lin, w_fc, b_fc):
    x = np.asarray(x, np.float32)
    src = np.asarray(edge_src, np.int64)
    dst = np.asarray(edge_dst, np.int64)
    ety = np.asarray(edge_type, np.int64)
    umask = np.asarray(umask, np.float32)
    basis = np.asarray(basis, np.float32)
    comp = np.asarray(comp, np.float32)
    w_root1 = np.asarray(w_root1, np.float32)
    b1 = np.asarray(b1, np.float32)
    w_rel2 = np.asarray(w_rel2, np.float32)
    b_rel2 = np.asarray(b_rel2, np.float32)
    w_root2 = np.asarray(w_root2, np.float32)
    w_t = np.asarray(w_t, np.float32)
    b_t = np.asarray(b_t, np.float32)
    w_lin = np.asarray(w_lin, np.float32)
    b_lin = np.asarray(b_lin, np.float32)
    w_fc = np.asarray(w_fc, np.float32)
    b_fc = np.asarray(b_fc, np.float32)

    # --- verify the structural assumptions the kernel exploits ---
    g_s = src // L
    assert np.array_equal(g_s, dst // L), "edges must stay within a dialogue"
    ls, ld = src % L, dst % L
    assert ((ld >= ls - WP) & (ld <= ls + WF)).all()
    spk = np.zeros(N, np.int64)
    spk[src] = ety >> 2
    spk[dst] = (ety >> 1) & 1
    dir_ = (ls >= ld).astype(np.int64)
    assert np.array_equal(ety, spk[src] * 4 + spk[dst] * 2 + dir_)
    cnt = np.zeros((B, L, L), np.int8)
    np.add.at(cnt, (g_s, ls, ld), 1)
    ii = np.arange(L)[:, None]
    kk = np.arange(L)[None, :]
    band = (kk >= ii - WP) & (kk <= ii + WF)
    assert (cnt == band[None].astype(np.int8)).all()

    deg = np.bincount(dst, minlength=N).astype(np.float64)[:L]
    invd = (1.0 / deg).astype(np.float32)
    W0 = (band & (ii < kk)).astype(np.float32)
    W1 = (band & (ii >= kk)).astype(np.float32)

    use_mask = not bool(np.all(umask == 1.0))

    # --- scales ---
    w_rel = np.einsum('rb,bdh->rdh', comp, basis)
    wlo = np.ascontiguousarray(w_rel[0:4].transpose(1, 0, 2).reshape(D, 512))
    whi = np.ascontiguousarray(w_rel[4:8].transpose(1, 0, 2).reshape(D, 512))

    sx = FP8MAX / float(np.abs(x).max())
    x8 = _q8(x * sx, "x")

    wdiff = wlo - whi
    swr = FP8MAX / max(float(np.abs(wdiff).max()), float(np.abs(whi).max()),
                       1e-30)
    wrd8 = _q8(wdiff * swr, "wrd")
    wrh8 = _q8(whi * swr, "wrh")

    rn_x8 = _rownorm_max(x8.astype(np.float32))
    cY_base = FP8MAX / (rn_x8 * (_colnorm_max(wrd8.astype(np.float32)) +
                                 _colnorm_max(wrh8.astype(np.float32))) * 1.07)
    SY_base = sx * swr * cY_base

    cn_wr1_base = _colnorm_max(w_root1) * swr * cY_base
    RBr = rn_x8 * cn_wr1_base * 1.07
    B1 = float(np.abs(b1).max())
    c1 = FP8MAX / ((FP8MAX + RBr + SY_base * B1) * 1.07)
    cY = cY_base * np.sqrt(c1)
    S1 = SY_base * c1
    wr18 = _q8(w_root1 * (swr * cY_base * c1), "wr1")

    m0 = _q8(W0 * (invd * np.sqrt(c1))[None, :], "aggm0")
    m1 = _q8(W1 * (invd * np.sqrt(c1))[None, :], "aggm1")
    b018 = _q8(W0 + W1, "b01")

    Snb = S1 / 32.0
    mx2a = float(np.abs(w_rel2).max()) or 1e-30
    mx2b = float(np.abs(w_root2).max()) or 1e-30
    s2a = FP8MAX / (1.07 * mx2a)
    s2b = s2a * Snb / S1
    if mx2b * s2b > FP8MAX:
        s2b = FP8MAX / (1.07 * mx2b)
        s2a = s2b * S1 / Snb
    w2a8 = (w_rel2 * s2a).astype(np.float32)
    w2b8 = (w_root2 * s2b).astype(np.float32)
    SP2 = s2a * Snb
    bound_o2 = np.sqrt(2 * H) * FP8MAX * max(
        _colnorm_max(w2a8.astype(np.float32)),
        _colnorm_max(w2b8.astype(np.float32))) * 1.07
    c2 = FP8MAX / bound_o2
    S2 = SP2 * c2

    mxwx = float(np.abs(w_t[:D]).max()) or 1e-30
    mxw2 = float(np.abs(w_t[D:]).max()) or 1e-30
    kt = min(FP8MAX / (1.07 * mxwx) * sx, FP8MAX / (1.07 * mxw2) * S2)
    wt8 = np.zeros((1280, MEM), np.float32)
    wt8[:D] = w_t[:D] * (kt / sx)
    wt8[D:MEM] = w_t[D:] * (kt / S2)
    wt8 = _q8(wt8, "wt")
    rn_M8 = float(np.sqrt(rn_x8 ** 2 + H * FP8MAX ** 2))
    bound_xc = rn_M8 * _colnorm_max(wt8.astype(np.float32)) * 1.07 \
        + kt * float(np.abs(b_t).max())
    s_Mxc = np.array([sx] * 8 + [S2], np.float64)
    CC = (FP8MAX / bound_xc) * s_Mxc.min()
    cXc = (CC / s_Mxc).astype(np.float32)
    Sscore = kt * CC

    mxlx = float(np.abs(w_lin[:D]).max()) or 1e-30
    mxl2 = float(np.abs(w_lin[D:]).max()) or 1e-30
    kg = min(FP8MAX / (1.07 * mxlx) * sx, FP8MAX / (1.07 * mxl2) * S2)
    wlin8 = np.zeros((1280, H), np.float32)
    wlin8[:D] = w_lin[:D] * (kg / sx)
    wlin8[D:MEM] = w_lin[D:] * (kg / S2)
    wlin8 = _q8(wlin8, "wlin")
    cg = FP8MAX / (rn_M8 * _colnorm_max(wlin8.astype(np.float32)) * 1.07)
    SG = kg * cg

    scal = np.zeros((128, 16), np.float32)
    scal[:, 0] = cY
    scal[:, 1] = c2
    scal[:, 2:11] = cXc[None, :]
    scal[:, 11] = 1.0 / Sscore
    scal[:, 12] = 1.0 / (64.0 * SG)
    scal[:, 13] = cg

    bias_pack = np.zeros((128, 12), np.float32)
    bias_pack[:, 0] = S1 * b1
    bias_pack[:, 1] = S2 * b_rel2
    bias_pack[:, 2:11] = (b_t.reshape(9, 128).T * (kt * cXc)[None, :])
    bias_pack[:, 11] = b_lin

    def pack_kp(a, width):
        k = a.shape[0] // 128
        return np.ascontiguousarray(a.reshape(k, 128, width).transpose(1, 0, 2))

    wt_pack = np.ascontiguousarray(
        wt8.reshape(MP, 128, MT, 128).transpose(1, 2, 0, 3))

    shared = {
        "wrd": pack_kp(wrd8, 512),
        "wrh": pack_kp(wrh8, 512),
        "wr1": pack_kp(wr18, H),
        "aggm": np.ascontiguousarray(np.stack(
            [m0.reshape(2, 128, L), m1.reshape(2, 128, L)], 0)
            .transpose(2, 0, 1, 3)),
        "b01": np.ascontiguousarray(b018.reshape(2, 128, L).transpose(1, 0, 2)),
        "w2": np.ascontiguousarray(np.stack([w2a8, w2b8], 0).transpose(1, 0, 2)).astype(BF16),
        "wt": wt_pack,
        "wlin": pack_kp(wlin8, H),
        "wfc": w_fc.astype(BF16),
        "bfc": b_fc.reshape(1, C).astype(BF16),
        "sc": scal,
        "bias": bias_pack,
    }

    spk_f = spk.astype(np.float32)
    in_maps = []
    for c in range(NCORES):
        nsl = slice(c * NLOC, (c + 1) * NLOC)
        m = dict(shared)
        xl8 = x8[nsl]
        m["xt"] = np.ascontiguousarray(
            xl8.T.reshape(KT, 128, NLOC).transpose(1, 0, 2))
        u0 = (1.0 - spk_f[nsl]).astype(np.float32)
        xu8 = (xl8.astype(np.float32) * u0[:, None]).astype(FP8)
        m["xu"] = np.ascontiguousarray(
            xu8.T.reshape(KT, 128, NLOC).transpose(1, 0, 2))
        m["vb"] = spk_f[nsl].reshape(DPC, L).astype(np.uint8)
        if use_mask:
            uml = umask[c * DPC:(c + 1) * DPC]
            m["um"] = np.broadcast_to(
                np.stack([uml * uml, uml], 1)[None], (128, DPC, 2, L)
            ).astype(np.float32).copy()
        in_maps.append(m)
    return in_maps, use_mask


_last_results = None


def kernel(**inputs):
    global _last_results
    from concourse.bass_utils import run_bass_kernel_spmd

    in_maps, use_mask = prep_inputs(**inputs)
    if use_mask not in _cache:
        _cache[use_mask] = _build_program(use_mask)
    nc = _cache[use_mask]
    res = run_bass_kernel_spmd(nc, in_maps, core_ids=list(range(NCORES)))
    _last_results = res
    return np.concatenate([res.results[c]["out"] for c in range(NCORES)], axis=0)


# revision 13
# speedup vs baseline: 1.0434x; 1.0434x over previous
"""Trainium2 Bass kernel for nn_DialogueGCNModel (DialogueGCN forward).

Strategy (data-parallel over dialogues, 4 dialogues per core, fp8 DoubleRow):
  - All large matmuls run in fp8e4 with MatmulPerfMode.DoubleRow (2x128
    contraction per instruction at 0.5 cycles/output-column).
  - The RGCN relational gather/scatter is factorized: etype = 4*spk_src +
    2*spk_dst + dir, so  agg = sum_{b,d} W'_d^T Y_{b,d}  with two SHARED
    banded direction masks W_d (invd and the out1 copy scale folded in).
    Y (the src-speaker select of xr) is built inside one psum group by
    matmul'ing a host-premasked copy xu = u0*x against (Wlo-Whi) and x
    against Whi; the dst-speaker select is one copy_predicated between two
    psum halves, and the root term accumulates into the same psum.
  - Every fp8 tensor is scaled on host via rigorous norm bounds so no cast
    can overflow; all unscale factors ride as [128,1] scalar-AP data so the
    compiled program is input-independent.
  - Host pre-packs every operand in its exact SBUF layout -> every DMA is a
    flat full-bandwidth copy, ordered by first use. Dialogues are paired in
    the psum free dim so psum->SBUF copies run at [128,512] granularity.

kernel(**inputs) takes FULL inputs, runs 8-core SPMD via
bass_utils.run_bass_kernel_spmd, returns the FULL (8192, 7) f32 output.
"""

import numpy as np
import ml_dtypes

BF16 = ml_dtypes.bfloat16
FP8 = ml_dtypes.float8_e4m3    # IEEE e4m3: max finite 240

B, L, D, H, R, NB, C = 32, 256, 1024, 128, 8, 30, 7
WP, WF = 10, 10
MEM = D + H            # 1152
N = B * L              # 8192
NCORES = 8
DPC = B // NCORES      # dialogues per core = 4
NLOC = DPC * L         # nodes per core = 1024
NT = NLOC // 128       # node tiles per core = 8
KT = D // 128          # contraction tiles over D = 8
MT = MEM // 128        # tiles over MEM = 9
MP = MT + 1            # padded to 1280 for DoubleRow pairing

FP8MAX = 112.0

_cache = {}


def _build_program(use_mask):
    import concourse.bacc as bacc
    import concourse.tile as tile
    import concourse.mybir as mybir
    import concourse.bass as bass
    from concourse.masks import make_identity

    dt = mybir.dt
    f32, bf16, fp8 = dt.float32, dt.bfloat16, dt.float8e4
    AX = mybir.AxisListType.X
    AF = mybir.ActivationFunctionType
    OP = mybir.AluOpType
    DR = mybir.MatmulPerfMode.DoubleRow

    nc = bacc.Bacc("TRN2", target_bir_lowering=False, debug=False,
                   num_devices=NCORES)

    dram = nc.dram_tensor
    xt_d = dram("xt", [128, KT, NLOC], fp8, kind="ExternalInput")
    xu_d = dram("xu", [128, KT, NLOC], fp8, kind="ExternalInput")
    wrd_d = dram("wrd", [128, KT, 512], fp8, kind="ExternalInput")
    wrh_d = dram("wrh", [128, KT, 512], fp8, kind="ExternalInput")
    wr1_d = dram("wr1", [128, KT, H], fp8, kind="ExternalInput")
    aggm_d = dram("aggm", [128, 2, 2, L], fp8, kind="ExternalInput")
    b01_d = dram("b01", [128, 2, L], fp8, kind="ExternalInput")
    w2_d = dram("w2", [128, 2, H], bf16, kind="ExternalInput")
    wt_d = dram("wt", [128, MT, MP, 128], fp8, kind="ExternalInput")
    wlin_d = dram("wlin", [128, MP, H], fp8, kind="ExternalInput")
    wfc_d = dram("wfc", [128, C], bf16, kind="ExternalInput")
    bfc_d = dram("bfc", [1, C], bf16, kind="ExternalInput")
    vb_d = dram("vb", [DPC, L], dt.uint8, kind="ExternalInput")
    sc_d = dram("sc", [128, 16], f32, kind="ExternalInput")
    bias_d = dram("bias", [128, 12], f32, kind="ExternalInput")
    if use_mask:
        um_d = dram("um", [128, DPC, 2, L], f32, kind="ExternalInput")
    out_d = dram("out", [NLOC, C], f32, kind="ExternalOutput")

    with tile.TileContext(nc) as tc:
        from contextlib import ExitStack
        with ExitStack() as ctx:
            consts = ctx.enter_context(tc.tile_pool(name="consts", bufs=1))
            work = ctx.enter_context(tc.tile_pool(name="work", bufs=6))
            ps = ctx.enter_context(tc.tile_pool(name="ps", bufs=6, space="PSUM"))
            pst = ctx.enter_context(tc.tile_pool(name="pst", bufs=1, space="PSUM"))

            dma = nc.sync.dma_start
            mm = nc.tensor.matmul

            xt = consts.tile([128, KT, NLOC], fp8)
            xu = consts.tile([128, KT, NLOC], fp8)
            wrd = consts.tile([128, KT, 512], fp8)
            wrh = consts.tile([128, KT, 512], fp8)
            wr1 = consts.tile([128, KT, H], fp8)
            aggm = consts.tile([128, 2, 2, L], fp8)
            b01 = consts.tile([128, 2, L], fp8)
            w2 = consts.tile([128, 2, H], bf16)
            wt = consts.tile([128, MT, MP, 128], fp8)
            wlin = consts.tile([128, MP, H], fp8)
            wfc = consts.tile([128, C], bf16)
            bfc = consts.tile([1, C], bf16)
            vb = consts.tile([128, DPC, L], dt.uint8)
            sc = consts.tile([128, 16], f32)
            bias = consts.tile([128, 12], f32)
            if use_mask:
                um = consts.tile([128, DPC, 2, L], f32)

            Y = consts.tile([128, NT, 512], fp8)
            rhs23 = consts.tile([128, 2, DPC, L], bf16)  # slot0 nbT, slot1 out1T
            o1n = consts.tile([128, DPC, 2, H], fp8)
            mtail = consts.tile([128, 2, DPC, L], fp8)   # slot0 out2T, slot1 zero
            XcT = consts.tile([128, MP, DPC, L], fp8)    # slot MT zero
            G8 = consts.tile([128, DPC, 2, H], fp8)
            alphaT = consts.tile([128, DPC, 2, L], fp8)
            z = consts.tile([128, DPC, 2, L], f32)
            hidT = consts.tile([128, DPC, L], bf16)
            o_all = consts.tile([128, DPC, 2, 8], f32)

            # DMAs in first-use order (SP engine queue)
            dma(out=wrh, in_=wrh_d[:])
            dma(out=xt[:, :, 0:128], in_=xt_d[:, :, 0:128])
            dma(out=wrd, in_=wrd_d[:])
            dma(out=xu[:, :, 0:128], in_=xu_d[:, :, 0:128])
            dma(out=xt[:, :, 128:NLOC], in_=xt_d[:, :, 128:NLOC])
            dma(out=xu[:, :, 128:NLOC], in_=xu_d[:, :, 128:NLOC])
            dma(out=sc, in_=sc_d[:])
            dma(out=bias, in_=bias_d[:])
            dma(out=aggm, in_=aggm_d[:])
            dma(out=wr1, in_=wr1_d[:])
            dma(out=b01, in_=b01_d[:])
            dma(out=w2, in_=w2_d[:])
            dma(out=wt, in_=wt_d[:])
            dma(out=wlin, in_=wlin_d[:])
            dma(out=wfc, in_=wfc_d[:])
            dma(out=bfc, in_=bfc_d[:])
            if use_mask:
                dma(out=um, in_=um_d[:])

            def bcast(dst, src_ap):
                bc = bass.AP(tensor=src_ap.tensor, offset=src_ap.offset,
                             ap=[[0, 128]] + list(src_ap.ap))
                nc.gpsimd.dma_start(out=dst, in_=bc)

            bcast(vb, vb_d[:])

            nc.scalar.add_instruction(mybir.InstLoadActFuncSet(
                name=nc.get_next_instruction_name(), ins=[], outs=[],
                act_func_set_id=6))
            ones_row = consts.tile([1, 128], bf16)
            nc.vector.memset(ones_row, 1.0)
            ident = consts.tile([128, 128], bf16)
            make_identity(nc, ident)
            nc.gpsimd.memset(mtail[:, 1, :, :], 0.0)
            nc.gpsimd.memset(XcT[:, MT, :, :], 0.0)

            # PE clock warm-up during the DMA lead-in
            warm_in = consts.tile([128, 512], bf16)
            nc.vector.memset(warm_in[:, 0:128], 0.0)
            warm = ps.tile([128, 2, 256], f32, tag="mm")
            for _ in range(16):
                mm(warm, lhsT=warm_in[:, 0:128], rhs=warm_in, start=True,
                   stop=True, skip_group_check=True)

            # ---- stage 1: Y psum = x @ Whi (+ u0*x @ (Wlo-Whi)) ----
            # sliding window: hi-group of tile i+1 is emitted before the
            # diff-group of tile i, giving the xu DMA extra slack
            p1s = {}

            def emit_hi(i):
                nsl = slice(i * 128, (i + 1) * 128)
                p1 = ps.tile([128, 2, 256], f32, tag="mm")
                p1s[i] = p1
                for k in range(KT // 2):
                    mm(p1[:, :, :], lhsT=xt[:, 2 * k:2 * k + 2, nsl],
                       rhs=wrh[:, 2 * k:2 * k + 2, :], start=(k == 0),
                       stop=False, perf_mode=DR, skip_group_check=True)

            def emit_diff(i):
                nsl = slice(i * 128, (i + 1) * 128)
                p1 = p1s.pop(i)
                for k in range(KT // 2):
                    mm(p1[:, :, :], lhsT=xu[:, 2 * k:2 * k + 2, nsl],
                       rhs=wrd[:, 2 * k:2 * k + 2, :], start=False,
                       stop=(k == KT // 2 - 1), perf_mode=DR,
                       skip_group_check=True)
                if i % 2 == 0:
                    nc.scalar.activation(Y[:, i, :], p1[:, :, :], AF.Copy,
                                         scale=sc[:, 0:1])
                else:
                    nc.vector.tensor_scalar(out=Y[:, i, :], in0=p1[:, :, :],
                                            scalar1=sc[:, 0:1], scalar2=None,
                                            op0=OP.mult)

            emit_hi(0)
            for i in range(1, NT):
                emit_hi(i)
                emit_diff(i - 1)
            emit_diff(NT - 1)

            # ---- stage 2: agg (pred-merge) + root, dialogue-paired ----
            pas, pbs = [], []
            for dp in range(DPC // 2):
                pa = ps.tile([128, 2, 256], f32, tag="mm")
                pb = ps.tile([128, 2, 256], f32, tag="mm")
                pas.append(pa)
                pbs.append(pb)
                for h in range(2):
                    d = 2 * dp + h
                    for dd in range(2):
                        mm(pa[:, h, :],
                           lhsT=Y[:, 2 * d:2 * d + 2, dd * H:(dd + 1) * H],
                           rhs=aggm[:, dd, :, :], start=(dd == 0), stop=False,
                           perf_mode=DR, skip_group_check=True)
                    for dd in range(2):
                        mm(pb[:, h, :],
                           lhsT=Y[:, 2 * d:2 * d + 2,
                                  (2 + dd) * H:(3 + dd) * H],
                           rhs=aggm[:, dd, :, :], start=(dd == 0),
                           stop=(dd == 1), perf_mode=DR, skip_group_check=True)
            for dp in range(DPC // 2):
                nc.vector.copy_predicated(pas[dp][:, :, :],
                                          vb[:, 2 * dp:2 * dp + 2, :],
                                          pbs[dp][:, :, :])
            for dp in range(DPC // 2):
                for h in range(2):
                    d = 2 * dp + h
                    for k in range(KT // 2):
                        mm(pas[dp][:, h, :], lhsT=wr1[:, 2 * k:2 * k + 2, :],
                           rhs=xt[:, 2 * k:2 * k + 2, d * L:(d + 1) * L],
                           start=False, stop=(k == KT // 2 - 1),
                           perf_mode=DR, skip_group_check=True)
            for dp in range(DPC // 2):
                nc.scalar.activation(
                    rhs23[:, 1, 2 * dp:2 * dp + 2, :], pas[dp][:, :, :],
                    AF.Identity, bias=bias[:, 0:1])

            # out1 transposes: [h, s] -> [s, h], 4 per d-pair in one psum
            for dp in range(DPC // 2):
                tp = pst.tile([128, 2, 256], bf16, tag="tr")
                for h in range(2):
                    d = 2 * dp + h
                    for st in range(2):
                        nc.tensor.transpose(
                            tp[:, h, st * 128:(st + 1) * 128],
                            rhs23[:, 1, d, st * 128:(st + 1) * 128], ident)
                nc.scalar.copy(o1n[:, 2 * dp:2 * dp + 2, :, :], tp)

            # ---- stage 3: GraphConv, dialogue-paired psums ----
            for dp in range(DPC // 2):
                p2 = ps.tile([128, 2, 256], f32, tag="mm")
                for h in range(2):
                    mm(p2[:, h, :], lhsT=o1n[:, 2 * dp + h, :, :], rhs=b01,
                       start=True, stop=True, perf_mode=DR,
                       skip_group_check=True)
                nc.vector.tensor_scalar(
                    out=rhs23[:, 0, 2 * dp:2 * dp + 2, :], in0=p2[:, :, :],
                    scalar1=0.03125, scalar2=None, op0=OP.mult)
            for dp in range(DPC // 2):
                p3 = ps.tile([128, 2, 256], f32, tag="mm")
                for h in range(2):
                    mm(p3[:, h, :], lhsT=w2[:, 0, :],
                       rhs=rhs23[:, 0, 2 * dp + h, :],
                       start=True, stop=False, skip_group_check=True)
                    mm(p3[:, h, :], lhsT=w2[:, 1, :],
                       rhs=rhs23[:, 1, 2 * dp + h, :],
                       start=False, stop=True, skip_group_check=True)
                nc.scalar.activation(mtail[:, 0, 2 * dp:2 * dp + 2, :],
                                     p3[:, :, :], AF.Identity,
                                     bias=bias[:, 1:2], scale=sc[:, 1:2])

            # M^T pair accessor for DoubleRow pair j of dialogue d
            def mpair(j, d, c0=0, c1=L):
                if j < 4:
                    return xt[:, 2 * j:2 * j + 2, d * L + c0:d * L + c1]
                return mtail[:, :, d, c0:c1]

            # ---- stages 5-6 pipelined per dialogue-pair: Xc -> scores
            # -> tanh/exp overlap the next pair's Xc psums on PE ----
            alfs = {}
            for dp in range(DPC // 2):
                for n2 in range(MT):
                    p4 = ps.tile([128, 2, 256], f32, tag="mm")
                    for h in range(2):
                        d = 2 * dp + h
                        for j in range(5):
                            mm(p4[:, h, :], lhsT=wt[:, n2, 2 * j:2 * j + 2, :],
                               rhs=mpair(j, d), start=(j == 0), stop=(j == 4),
                               perf_mode=DR, skip_group_check=True)
                    if n2 % 2 == 0:
                        nc.scalar.activation(
                            XcT[:, n2, 2 * dp:2 * dp + 2, :], p4[:, :, :],
                            AF.Identity, bias=bias[:, 2 + n2:3 + n2],
                            scale=sc[:, 2 + n2:3 + n2])
                    else:
                        nc.vector.tensor_scalar(
                            out=XcT[:, n2, 2 * dp:2 * dp + 2, :],
                            in0=p4[:, :, :], scalar1=sc[:, 2 + n2:3 + n2],
                            scalar2=bias[:, 2 + n2:3 + n2],
                            op0=OP.mult, op1=OP.add)
                # scores + tanh for this pair
                for h in range(2):
                    d = 2 * dp + h
                    p5 = ps.tile([128, 2, 256], f32, tag="mm")
                    for tt in range(2):
                        for j in range(5):
                            mm(p5[:, tt, :],
                               lhsT=XcT[:, 2 * j:2 * j + 2, d,
                                        tt * 128:(tt + 1) * 128],
                               rhs=mpair(j, d), start=(j == 0), stop=(j == 4),
                               perf_mode=DR, skip_group_check=True)
                    # tanh-free: z = 1 - 2/(1 + e^{2s}); keeps every ACT
                    # function in the natural_log_exp set (no table reloads)
                    if use_mask:
                        nc.vector.scalar_tensor_tensor(
                            out=z[:, d, :, :], in0=p5[:, :, :],
                            scalar=sc[:, 14:15], in1=um[:, d, :, :],
                            op0=OP.mult, op1=OP.mult)
                        nc.scalar.activation(z[:, d, :, :], z[:, d, :, :],
                                             AF.Exp)
                    else:
                        nc.scalar.activation(z[:, d, :, :], p5[:, :, :],
                                             AF.Exp, scale=sc[:, 14:15])
                    nc.vector.tensor_scalar(
                        out=z[:, d, :, :], in0=z[:, d, :, :], scalar1=1.0,
                        scalar2=None, op0=OP.add)
                    nc.vector.reciprocal(z[:, d, :, :], z[:, d, :, :])
                # G psums for this pair (keeps PE fed during tanh/exp)
                for h in range(2):
                    d = 2 * dp + h
                    pg = ps.tile([128, 2, 256], f32, tag="mm")
                    for st in range(2):
                        for j in range(5):
                            mm(pg[:, st, 0:128],
                               lhsT=mpair(j, d, st * 128, (st + 1) * 128),
                               rhs=wlin[:, 2 * j:2 * j + 2, :], start=(j == 0),
                               stop=(j == 4), perf_mode=DR,
                               skip_group_check=True)
                    nc.vector.tensor_scalar(out=G8[:, d, :, :],
                                            in0=pg[:, :, 0:128],
                                            scalar1=sc[:, 13:14], scalar2=None,
                                            op0=OP.mult)
                # softmax exp/recip/alpha for this pair
                for h in range(2):
                    d = 2 * dp + h
                    for tt in range(2):
                        zz = z[:, d, tt, :]
                        ssum = work.tile([128, 1], f32, tag=f"ssum{d}{tt}")
                        nc.scalar.activation(zz, zz, AF.Exp, bias=1.0,
                                             scale=-2.0, accum_out=ssum)
                        if use_mask:
                            nc.vector.tensor_mul(zz, zz, um[:, d, 1, :])
                            nc.vector.reduce_sum(out=ssum, in_=zz, axis=AX)
                        rinv = work.tile([128, 1], f32, tag=f"rinv{d}{tt}")
                        nc.vector.reciprocal(rinv, ssum)
                        alf = work.tile([128, L], bf16, tag=f"alf{d}{tt}")
                        nc.gpsimd.tensor_scalar(out=alf, in0=zz, scalar1=rinv,
                                                scalar2=64.0, op0=OP.mult,
                                                op1=OP.mult)
                        alfs[(d, tt)] = alf

            # alpha transposes (PE) + copies (DVE)
            for d in range(DPC):
                tpa = pst.tile([128, 2, 256], bf16, tag="tr")
                for st in range(2):
                    for tt in range(2):
                        nc.tensor.transpose(
                            tpa[:, st, tt * 128:(tt + 1) * 128],
                            alfs[(d, tt)][:, st * 128:(st + 1) * 128], ident)
                if d % 2 == 0:
                    nc.vector.tensor_copy(alphaT[:, d, :, :], tpa)
                else:
                    nc.scalar.copy(alphaT[:, d, :, :], tpa)
            # hidden psums (PE) + Relu (ACT)
            for d in range(DPC):
                p7 = ps.tile([128, 2, 256], f32, tag="mm")
                mm(p7[:, 0, :], lhsT=G8[:, d, :, :], rhs=alphaT[:, d, :, :],
                   start=True, stop=True, perf_mode=DR, skip_group_check=True)
                nc.scalar.activation(hidT[:, d, :], p7[:, 0, :], AF.Relu,
                                     bias=bias[:, 11:12], scale=sc[:, 12:13])
            # logits psums (PE) + Exp (ACT)
            s7s = {}
            for d in range(DPC):
                for tt in range(2):
                    p8 = ps.tile([128, 2, 256], f32, tag="mm")
                    mm(p8[:, 0, :C], lhsT=hidT[:, d, tt * 128:(tt + 1) * 128],
                       rhs=wfc, start=True, stop=False, skip_group_check=True)
                    mm(p8[:, 0, :C], lhsT=ones_row, rhs=bfc, start=False,
                       stop=True, skip_group_check=True)
                    e7 = work.tile([128, 8], f32, tag="e7")
                    s7 = work.tile([128, 1], f32, tag=f"s7_{d}{tt}")
                    nc.scalar.activation(e7[:, :C], p8[:, 0, :C], AF.Exp,
                                         accum_out=s7)
                    nc.vector.tensor_scalar(
                        out=o_all[:, d, tt, :C], in0=p8[:, 0, :C],
                        scalar1=0.0, scalar2=None, op0=OP.add)
                    s7s[(d, tt)] = s7
            # Ln (one table switch) + subtract (DVE)
            for d in range(DPC):
                for tt in range(2):
                    ls7 = work.tile([128, 1], f32, tag=f"ls7_{d}{tt}")
                    nc.scalar.activation(ls7, s7s[(d, tt)], AF.Ln)
                    nc.vector.tensor_scalar(
                        out=o_all[:, d, tt, :C], in0=o_all[:, d, tt, :C],
                        scalar1=ls7, scalar2=None, op0=OP.subtract)

            dma(out=out_d[:].rearrange("(d tt p) c -> p d tt c", d=DPC, tt=2),
                in_=o_all[:, :, :, 0:C])

    nc.compile()
    return nc


def _colnorm_max(a):
    return float(np.sqrt((np.asarray(a, np.float64) ** 2).sum(0)).max())


def _rownorm_max(a):
    return float(np.sqrt((np.asarray(a, np.float64) ** 2).sum(-1)).max())


def _q8(a, name=""):
    a = np.asarray(a, np.float32)
    mx = float(np.abs(a).max()) if a.size else 0.0
    assert mx <= 239.0, f"fp8 overflow in {name}: {mx}"
    return a.astype(FP8)


def prep_inputs(x, edge_src, edge_dst, edge_type, umask, basis, comp,
                w_root1, b1, w_rel2, b_rel2, w_root2, w_t, b_t,
                w_lin, b_lin, w_fc, b_fc):
    x = np.asarray(x, np.float32)
    src = np.asarray(edge_src, np.int64)
    dst = np.asarray(edge_dst, np.int64)
    ety = np.asarray(edge_type, np.int64)
    umask = np.asarray(umask, np.float32)
    basis = np.asarray(basis, np.float32)
    comp = np.asarray(comp, np.float32)
    w_root1 = np.asarray(w_root1, np.float32)
    b1 = np.asarray(b1, np.float32)
    w_rel2 = np.asarray(w_rel2, np.float32)
    b_rel2 = np.asarray(b_rel2, np.float32)
    w_root2 = np.asarray(w_root2, np.float32)
    w_t = np.asarray(w_t, np.float32)
    b_t = np.asarray(b_t, np.float32)
    w_lin = np.asarray(w_lin, np.float32)
    b_lin = np.asarray(b_lin, np.float32)
    w_fc = np.asarray(w_fc, np.float32)
    b_fc = np.asarray(b_fc, np.float32)

    # --- verify the structural assumptions the kernel exploits ---
    g_s = src // L
    assert np.array_equal(g_s, dst // L), "edges must stay within a dialogue"
    ls, ld = src % L, dst % L
    assert ((ld >= ls - WP) & (ld <= ls + WF)).all()
    spk = np.zeros(N, np.int64)
    spk[src] = ety >> 2
    spk[dst] = (ety >> 1) & 1
    dir_ = (ls >= ld).astype(np.int64)
    assert np.array_equal(ety, spk[src] * 4 + spk[dst] * 2 + dir_)
    cnt = np.zeros((B, L, L), np.int8)
    np.add.at(cnt, (g_s, ls, ld), 1)
    ii = np.arange(L)[:, None]
    kk = np.arange(L)[None, :]
    band = (kk >= ii - WP) & (kk <= ii + WF)
    assert (cnt == band[None].astype(np.int8)).all()

    deg = np.bincount(dst, minlength=N).astype(np.float64)[:L]
    invd = (1.0 / deg).astype(np.float32)
    W0 = (band & (ii < kk)).astype(np.float32)
    W1 = (band & (ii >= kk)).astype(np.float32)

    use_mask = not bool(np.all(umask == 1.0))

    # --- scales ---
    w_rel = np.einsum('rb,bdh->rdh', comp, basis)
    wlo = np.ascontiguousarray(w_rel[0:4].transpose(1, 0, 2).reshape(D, 512))
    whi = np.ascontiguousarray(w_rel[4:8].transpose(1, 0, 2).reshape(D, 512))

    sx = FP8MAX / float(np.abs(x).max())
    x8 = _q8(x * sx, "x")

    wdiff = wlo - whi
    swr = FP8MAX / max(float(np.abs(wdiff).max()), float(np.abs(whi).max()),
                       1e-30)
    wrd8 = _q8(wdiff * swr, "wrd")
    wrh8 = _q8(whi * swr, "wrh")

    rn_x8 = _rownorm_max(x8.astype(np.float32))
    cY_base = FP8MAX / (rn_x8 * (_colnorm_max(wrd8.astype(np.float32)) +
                                 _colnorm_max(wrh8.astype(np.float32))) * 1.07)
    SY_base = sx * swr * cY_base

    cn_wr1_base = _colnorm_max(w_root1) * swr * cY_base
    RBr = rn_x8 * cn_wr1_base * 1.07
    B1 = float(np.abs(b1).max())
    c1 = FP8MAX / ((FP8MAX + RBr + SY_base * B1) * 1.07)
    cY = cY_base * np.sqrt(c1)
    S1 = SY_base * c1
    wr18 = _q8(w_root1 * (swr * cY_base * c1), "wr1")

    m0 = _q8(W0 * (invd * np.sqrt(c1))[None, :], "aggm0")
    m1 = _q8(W1 * (invd * np.sqrt(c1))[None, :], "aggm1")
    b018 = _q8(W0 + W1, "b01")

    Snb = S1 / 32.0
    mx2a = float(np.abs(w_rel2).max()) or 1e-30
    mx2b = float(np.abs(w_root2).max()) or 1e-30
    s2a = FP8MAX / (1.07 * mx2a)
    s2b = s2a * Snb / S1
    if mx2b * s2b > FP8MAX:
        s2b = FP8MAX / (1.07 * mx2b)
        s2a = s2b * S1 / Snb
    w2a8 = (w_rel2 * s2a).astype(np.float32)
    w2b8 = (w_root2 * s2b).astype(np.float32)
    SP2 = s2a * Snb
    bound_o2 = np.sqrt(2 * H) * FP8MAX * max(
        _colnorm_max(w2a8.astype(np.float32)),
        _colnorm_max(w2b8.astype(np.float32))) * 1.07
    c2 = FP8MAX / bound_o2
    S2 = SP2 * c2

    mxwx = float(np.abs(w_t[:D]).max()) or 1e-30
    mxw2 = float(np.abs(w_t[D:]).max()) or 1e-30
    kt = min(FP8MAX / (1.07 * mxwx) * sx, FP8MAX / (1.07 * mxw2) * S2)
    wt8 = np.zeros((1280, MEM), np.float32)
    wt8[:D] = w_t[:D] * (kt / sx)
    wt8[D:MEM] = w_t[D:] * (kt / S2)
    wt8 = _q8(wt8, "wt")
    rn_M8 = float(np.sqrt(rn_x8 ** 2 + H * FP8MAX ** 2))
    bound_xc = rn_M8 * _colnorm_max(wt8.astype(np.float32)) * 1.07 \
        + kt * float(np.abs(b_t).max())
    s_Mxc = np.array([sx] * 8 + [S2], np.float64)
    CC = (FP8MAX / bound_xc) * s_Mxc.min()
    cXc = (CC / s_Mxc).astype(np.float32)
    Sscore = kt * CC

    mxlx = float(np.abs(w_lin[:D]).max()) or 1e-30
    mxl2 = float(np.abs(w_lin[D:]).max()) or 1e-30
    kg = min(FP8MAX / (1.07 * mxlx) * sx, FP8MAX / (1.07 * mxl2) * S2)
    wlin8 = np.zeros((1280, H), np.float32)
    wlin8[:D] = w_lin[:D] * (kg / sx)
    wlin8[D:MEM] = w_lin[D:] * (kg / S2)
    wlin8 = _q8(wlin8, "wlin")
    cg = FP8MAX / (rn_M8 * _colnorm_max(wlin8.astype(np.float32)) * 1.07)
    SG = kg * cg

    scal = np.zeros((128, 16), np.float32)
    scal[:, 0] = cY
    scal[:, 1] = c2
    scal[:, 2:11] = cXc[None, :]
    scal[:, 11] = 1.0 / Sscore
    scal[:, 14] = 2.0 / Sscore
    scal[:, 12] = 1.0 / (64.0 * SG)
    scal[:, 13] = cg

    bias_pack = np.zeros((128, 12), np.float32)
    bias_pack[:, 0] = S1 * b1
    bias_pack[:, 1] = S2 * b_rel2
    bias_pack[:, 2:11] = (b_t.reshape(9, 128).T * (kt * cXc)[None, :])
    bias_pack[:, 11] = b_lin

    def pack_kp(a, width):
        k = a.shape[0] // 128
        return np.ascontiguousarray(a.reshape(k, 128, width).transpose(1, 0, 2))

    wt_pack = np.ascontiguousarray(
        wt8.reshape(MP, 128, MT, 128).transpose(1, 2, 0, 3))

    shared = {
        "wrd": pack_kp(wrd8, 512),
        "wrh": pack_kp(wrh8, 512),
        "wr1": pack_kp(wr18, H),
        "aggm": np.ascontiguousarray(np.stack(
            [m0.reshape(2, 128, L), m1.reshape(2, 128, L)], 0)
            .transpose(2, 0, 1, 3)),
        "b01": np.ascontiguousarray(b018.reshape(2, 128, L).transpose(1, 0, 2)),
        "w2": np.ascontiguousarray(np.stack([w2a8, w2b8], 0).transpose(1, 0, 2)).astype(BF16),
        "wt": wt_pack,
        "wlin": pack_kp(wlin8, H),
        "wfc": w_fc.astype(BF16),
        "bfc": b_fc.reshape(1, C).astype(BF16),
        "sc": scal,
        "bias": bias_pack,
    }

    spk_f = spk.astype(np.float32)
    in_maps = []
    for c in range(NCORES):
        nsl = slice(c * NLOC, (c + 1) * NLOC)
        m = dict(shared)
        xl8 = x8[nsl]
        m["xt"] = np.ascontiguousarray(
            xl8.T.reshape(KT, 128, NLOC).transpose(1, 0, 2))
        u0 = (1.0 - spk_f[nsl]).astype(np.float32)
        xu8 = (xl8.astype(np.float32) * u0[:, None]).astype(FP8)
        m["xu"] = np.ascontiguousarray(
            xu8.T.reshape(KT, 128, NLOC).transpose(1, 0, 2))
        m["vb"] = spk_f[nsl].reshape(DPC, L).astype(np.uint8)
        if use_mask:
            uml = umask[c * DPC:(c + 1) * DPC]
            m["um"] = np.broadcast_to(
                np.stack([uml * uml, uml], 1)[None], (128, DPC, 2, L)
            ).astype(np.float32).copy()
        in_maps.append(m)
    return in_maps, use_mask


_last_results = None


def kernel(**inputs):
    global _last_results
    from concourse.bass_utils import run_bass_kernel_spmd

    in_maps, use_mask = prep_inputs(**inputs)
    if use_mask not in _cache:
        _cache[use_mask] = _build_program(use_mask)
    nc = _cache[use_mask]
    res = run_bass_kernel_spmd(nc, in_maps, core_ids=list(range(NCORES)))
    _last_results = res
    return np.concatenate([res.results[c]["out"] for c in range(NCORES)], axis=0)
